# revision 1
# baseline (speedup 1.0000x reference)
"""Trainium2 Bass kernel for combined cross-entropy + batch-hard triplet loss.

Problem (N=4096, C=751, D=2048, 1024 identities x 4 instances):
  loss = mean(-log_softmax(logits)[i, t_i]) +
         mean(relu(max_same(dist) - min_diff(dist) + 0.5))
  with dist = pairwise Euclidean distances of feat rows.

Sharding: row-parallel. Core c computes the [512, 4096] block of the Gram
matrix for its rows via fp32r matmuls (FP22 multiply precision, full PE rate),
with -sq_i/2 - sq_j/2 - 65536*same(i,j) folded into the same PSUM accumulation
through one extra K=36 matmul per block:
  rows 0..31 : 256 * onehot(group_of(i))   x   -256 * onehot(group_of(j))
  row  32,33 : 1                           x   -sq_hi[j]/2 , -sq_lo[j]/2
  row  34,35 : -sq_hi[i]/2 , -sq_lo[i]/2   x   1
(256 = 2^8 and the hi/lo split keep everything exactly representable in FP22.)

Then per row: q = -2*psum = d2 + 131072*same, so
  hardest-negative^2 = -2 * max_j(psum)          (same-entries pushed far down)
  hardest-positive^2 = -2 * min_j(psum) - 131072 (same-entries pushed far up...
                        i.e. min picks the masked same entries)
Both come from plain DVE reduces directly on PSUM. Rows are pre-sorted by
target on the host (the loss is invariant to row permutation), which makes the
same-identity mask a fixed block-diagonal pattern of 4-row groups.

Each core also handles the cross entropy for its 512 rows (ACT exp with fused
row-sum, one-hot gather via scalar_tensor_tensor). Output per core: [128, 8]
(4 cols of per-row xent terms, 4 cols of per-row triplet terms, one col per
128-row tile). Host sums and averages.
"""

import sys

if "/opt/trn_rl_repo" not in sys.path:
    sys.path.insert(0, "/opt/trn_rl_repo")

import numpy as np

N = 4096
D = 2048
C = 751
NCORES = 8
RPC = N // NCORES          # rows per core = 512
MT = RPC // 128            # 128-row tiles per core = 4
NB = N // 512              # 512-wide column blocks = 8
KT = D // 128              # K chunks = 16
KF = 36                    # fold matmul contraction size
BIG = 131072.0             # 2^17: same-pair offset in q = -2*psum
MASK_SCALE = 256.0         # sqrt(BIG/2) = 2^8, exact in FP22
ALPHA = 1.0
BETA = 1.0
MARGIN = 0.5

_compiled = {}


def _build_nc():
    import concourse.bass as bass  # noqa: F401
    import concourse.tile as tile
    from concourse import mybir, bacc
    from contextlib import ExitStack

    f32 = mybir.dt.float32
    f32r = mybir.dt.float32r
    Alu = mybir.AluOpType
    Act = mybir.ActivationFunctionType
    X = mybir.AxisListType.X

    nc = bacc.Bacc("TRN2", target_bir_lowering=False, debug=False)

    fT = nc.dram_tensor("fT", [D, N], f32r, kind="ExternalInput").ap()
    lhsT = nc.dram_tensor("lhsT", [D, RPC], f32r, kind="ExternalInput").ap()
    fold_rhs = nc.dram_tensor("fold_rhs", [MT * KF, N], f32r, kind="ExternalInput").ap()
    fold_lhsT = nc.dram_tensor("fold_lhsT", [KF, RPC], f32r, kind="ExternalInput").ap()
    logits_in = nc.dram_tensor("logits", [RPC, C], f32, kind="ExternalInput").ap()
    onehot_in = nc.dram_tensor("onehot", [RPC, C], f32, kind="ExternalInput").ap()
    consts_in = nc.dram_tensor("consts", [128, 2], f32, kind="ExternalInput").ap()
    out_dram = nc.dram_tensor("out", [128, 8], f32, kind="ExternalOutput").ap()

    with tile.TileContext(nc) as tc, ExitStack() as ctx:
        resident = ctx.enter_context(tc.tile_pool(name="resident", bufs=1))
        rhs_pool = ctx.enter_context(tc.tile_pool(name="rhs", bufs=32))
        fold_pool = ctx.enter_context(tc.tile_pool(name="fold", bufs=6))
        psum_pool = ctx.enter_context(tc.tile_pool(name="psum", bufs=8, space="PSUM"))
        xent_pool = ctx.enter_context(tc.tile_pool(name="xent", bufs=2))
        small_pool = ctx.enter_context(tc.tile_pool(name="small", bufs=2))

        # --- resident data ---
        lhsT_all = resident.tile([128, KT * RPC], f32r)   # k-chunk k at cols [RPC*k, RPC*(k+1))
        for k in range(KT):
            nc.sync.dma_start(lhsT_all[:, bass.ts(k, RPC)], lhsT[bass.ts(k, 128), :])
        flh = resident.tile([KF, RPC], f32r)
        nc.sync.dma_start(flh[:], fold_lhsT[:])
        consts = resident.tile([128, 2], f32)
        nc.sync.dma_start(consts[:], consts_in[:])
        neg_big = consts[:, 0:1]
        margin = consts[:, 1:2]

        mx_slots = [resident.tile([128, NB], f32, tag=f"mxs{m}", name=f"mxs{m}") for m in range(MT)]
        mn_slots = [resident.tile([128, NB], f32, tag=f"mns{m}", name=f"mns{m}") for m in range(MT)]
        out_tile = resident.tile([128, 8], f32)

        # --- main GEMM + mining ---
        for n in range(NB):
            rhs_tiles = []
            for k in range(KT):
                rt = rhs_pool.tile([128, 512], f32r, tag="rhs")
                nc.sync.dma_start(rt[:], fT[bass.ts(k, 128), bass.ts(n, 512)])
                rhs_tiles.append(rt)
            for m in range(MT):
                fr = fold_pool.tile([KF, 512], f32r, tag="fr")
                nc.sync.dma_start(fr[:], fold_rhs[bass.ds(m * KF, KF), bass.ts(n, 512)])
                ps = psum_pool.tile([128, 512], mybir.dt.float32, tag="ps")
                for k in range(KT):
                    nc.tensor.matmul(
                        ps[:],
                        lhsT_all[:, bass.ds(RPC * k + 128 * m, 128)],
                        rhs_tiles[k][:],
                        start=(k == 0),
                        stop=False,
                    )
                nc.tensor.matmul(ps[:], flh[:, bass.ts(m, 128)], fr[:],
                                 start=False, stop=True)
                nc.vector.tensor_reduce(mx_slots[m][:, n:n + 1], ps[:], axis=X, op=Alu.max)
                nc.vector.tensor_reduce(mn_slots[m][:, n:n + 1], ps[:], axis=X, op=Alu.min)

        # --- triplet tails ---
        for m in range(MT):
            t_an = small_pool.tile([128, 1], f32, tag="t_an")
            t_ap = small_pool.tile([128, 1], f32, tag="t_ap")
            nc.vector.tensor_reduce(t_an[:], mx_slots[m][:], axis=X, op=Alu.max)
            nc.vector.tensor_reduce(t_ap[:], mn_slots[m][:], axis=X, op=Alu.min)
            d_an = small_pool.tile([128, 1], f32, tag="d_an")
            d_ap = small_pool.tile([128, 1], f32, tag="d_ap")
            # dist_an = sqrt(-2 * t_an) ; dist_ap = sqrt(-2 * t_ap - BIG)
            nc.scalar.activation(d_an[:], t_an[:], Act.Sqrt, scale=-2.0)
            nc.scalar.activation(d_ap[:], t_ap[:], Act.Sqrt, bias=neg_big, scale=-2.0)
            diff = small_pool.tile([128, 1], f32, tag="diff")
            nc.vector.tensor_sub(diff[:], d_ap[:], d_an[:])
            nc.scalar.activation(out_tile[:, 4 + m:5 + m], diff[:], Act.Relu,
                                 bias=margin, scale=1.0)

        # --- cross entropy ---
        for r in range(MT):
            lg = xent_pool.tile([128, C], f32, tag="lg")
            oh = xent_pool.tile([128, C], f32, tag="oh")
            nc.sync.dma_start(lg[:], logits_in[bass.ts(r, 128), :])
            nc.sync.dma_start(oh[:], onehot_in[bass.ts(r, 128), :])
            mx = small_pool.tile([128, 1], f32, tag="xmx")
            nc.vector.tensor_reduce(mx[:], lg[:], axis=X, op=Alu.max)
            negmx = small_pool.tile([128, 1], f32, tag="negmx")
            nc.vector.tensor_scalar_mul(negmx[:], mx[:], -1.0)
            escr = xent_pool.tile([128, C], f32, tag="escr")
            s = small_pool.tile([128, 1], f32, tag="s")
            nc.scalar.activation(escr[:], lg[:], Act.Exp, bias=negmx[:], scale=1.0,
                                 accum_out=s[:])
            gscr = xent_pool.tile([128, C], f32, tag="gscr")
            tv = small_pool.tile([128, 1], f32, tag="tv")
            nc.vector.scalar_tensor_tensor(out=gscr[:], in0=lg[:], scalar=1.0,
                                           in1=oh[:], op0=Alu.mult, op1=Alu.mult,
                                           accum_out=tv[:])
            l1 = small_pool.tile([128, 1], f32, tag="l1")
            nc.scalar.activation(l1[:], s[:], Act.Ln, scale=1.0)
            # xent_row = (l1 + mx) - tv
            nc.vector.scalar_tensor_tensor(out=out_tile[:, r:r + 1], in0=l1[:],
                                           scalar=mx[:], in1=tv[:],
                                           op0=Alu.add, op1=Alu.subtract)

        nc.sync.dma_start(out_dram[:], out_tile[:])

    nc.compile()
    return nc


def _fp22_hi(v):
    return (np.ascontiguousarray(v, dtype=np.float32).view(np.uint32)
            & np.uint32(0xFFFFFC00)).view(np.float32)


def _prepare(logits, feat, targets):
    logits = np.asarray(logits, dtype=np.float32)
    feat = np.asarray(feat, dtype=np.float32)
    targets = np.asarray(targets)

    perm = np.argsort(targets, kind="stable")
    t = np.asarray(targets)[perm]
    tg = t.reshape(-1, 4)
    assert (tg == tg[:, :1]).all(), "expected PK sampling with 4 instances/identity"

    feat_p = feat[perm]
    logits_p = logits[perm]

    fT = np.ascontiguousarray(feat_p.T)                      # [D, N]
    sq = np.einsum("ij,ij->i", feat_p.astype(np.float64), feat_p.astype(np.float64))
    sq = sq.astype(np.float32)
    sq_hi = _fp22_hi(sq)
    sq_lo = (sq - sq_hi).astype(np.float32)

    # fold_lhsT [KF, RPC] per core: rows 0..31 structural mask (identical for
    # every core), rows 32,33 ones, rows 34,35 -sq_hi/2, -sq_lo/2 of own rows.
    mask_pat = np.zeros((32, RPC), dtype=np.float32)
    idx = np.arange(RPC)
    mask_pat[(idx % 128) // 4, idx] = MASK_SCALE

    in_maps = []
    for c in range(NCORES):
        rows = slice(c * RPC, (c + 1) * RPC)
        flh = np.zeros((KF, RPC), dtype=np.float32)
        flh[:32] = mask_pat
        flh[32] = 1.0
        flh[33] = 1.0
        flh[34] = -0.5 * sq_hi[rows]
        flh[35] = -0.5 * sq_lo[rows]

        frh = np.zeros((MT * KF, N), dtype=np.float32)
        for m in range(MT):
            blk = frh[m * KF:(m + 1) * KF]
            # group g of m-tile m covers columns c*RPC + 128*m + 4*g ... +4
            base = c * RPC + 128 * m
            for g in range(32):
                blk[g, base + 4 * g: base + 4 * g + 4] = -MASK_SCALE
            blk[32] = -0.5 * sq_hi
            blk[33] = -0.5 * sq_lo
            blk[34] = 1.0
            blk[35] = 1.0

        # match jax gather semantics: negative indices wrap, then clamp
        ti = t[rows].astype(np.int64)
        ti = np.where(ti < 0, ti + C, ti)
        ti = np.clip(ti, 0, C - 1)
        oh = np.zeros((RPC, C), dtype=np.float32)
        oh[np.arange(RPC), ti] = 1.0

        consts = np.zeros((128, 2), dtype=np.float32)
        consts[:, 0] = -BIG
        consts[:, 1] = MARGIN

        in_maps.append({
            "fT": fT,
            "lhsT": np.ascontiguousarray(fT[:, rows]),
            "fold_rhs": frh,
            "fold_lhsT": flh,
            "logits": np.ascontiguousarray(logits_p[rows]),
            "onehot": oh,
            "consts": consts,
        })
    return in_maps


def _combine(results):
    xent_sum = 0.0
    trip_sum = 0.0
    for r in results:
        o = r["out"].astype(np.float64)
        xent_sum += o[:, :4].sum()
        trip_sum += o[:, 4:].sum()
    loss = ALPHA * (xent_sum / N) + BETA * (trip_sum / N)
    return np.float32(loss)


def kernel(logits, feat, targets):
    from concourse.bass_utils import run_bass_kernel_spmd

    if "nc" not in _compiled:
        _compiled["nc"] = _build_nc()
    nc = _compiled["nc"]

    in_maps = _prepare(logits, feat, targets)
    res = run_bass_kernel_spmd(nc, in_maps, core_ids=list(range(NCORES)))
    return _combine(res.results)



# revision 4
# speedup vs baseline: 1.8293x; 1.8293x over previous
"""Trainium2 Bass kernel for combined cross-entropy + batch-hard triplet loss.

Problem (N=4096, C=751, D=2048, 1024 identities x 4 instances):
  loss = mean(-log_softmax(logits)[i, t_i]) +
         mean(relu(max_same(dist) - min_diff(dist) + 0.5))
  with dist = pairwise Euclidean distances of feat rows.

v2 design (row-parallel over 8 cores, 512 rows each):
- feat is quantized to fp8e4m3 on the host; the Gram matrix block
  [512, 4096] is computed with DoubleRow fp8 matmuls (2 K-chunks of 128
  per instruction, 2x PE rate). sq_i is recomputed from the QUANTIZED
  features so d2 = sq_i + sq_j - 2*G is exactly the distance matrix of
  the quantized features (error vs fp32 reference ~2e-2 absolute on
  d~64, far inside the 2e-2 relative tolerance on the scalar loss).
- Stationary-weight reuse: for each (m-tile, k-pair) the weight load is
  shared by all 8 column blocks (psum banks 0..7 hold one full m-row of
  the Gram block), cutting LDWEIGHTS traffic 8x vs one-load-per-matmul.
- A K=34 bf16 "fold" matmul adds -sq_j/2 (split hi/lo for exactness)
  and -65536 on same-identity pairs (rows pre-sorted by target on the
  host -> block-diagonal 4-row groups; the mask data is per-core, the
  program is uniform across cores).
- Mining on device: row max of psum over all 4096 cols -> hardest
  negative; row min over the m-tile's own 128-col diagonal window ->
  hardest positive (the -65536 mask guarantees the masked entries win
  the min). The sqrt/relu/margin tail runs on the host (4096 rows).
- Cross entropy: logits in bf16, device computes row max and
  sum(exp(l - max)) via one ACT Exp with fused accumulation; host does
  ln, adds the target logit (host gather) and averages.

Per-core output [128, 16]: cols 0..3 row-max(psum) per m-tile, 4..7
row-min(window) per m-tile, 8..11 logits row max, 12..15 exp sums.
"""

import os
import sys

if "/opt/trn_rl_repo" not in sys.path:
    sys.path.insert(0, "/opt/trn_rl_repo")

import numpy as np
import ml_dtypes

N = 4096
D = 2048
C = 751
NCORES = 8
RPC = N // NCORES          # rows per core = 512
MT = RPC // 128            # 128-row tiles per core = 4
NB = N // 512              # 512-wide column blocks = 8
KT = D // 128              # 128-row contraction chunks = 16
KF = 34                    # fold contraction: 2 sq rows + 32 mask rows
BIG = 131072.0             # 2^17 offset on same pairs in q = -2*psum
MASK_SCALE = 256.0         # 2^8, exact in bf16/fp8
ALPHA = 1.0
BETA = 1.0
MARGIN = 0.5

GRAM_MODE = os.environ.get("GRAM_MODE", "fp8")   # "fp8" | "bf16"

_compiled = {}


def _build_nc():
    import concourse.bass as bass  # noqa: F401
    import concourse.tile as tile
    from concourse import mybir, bacc
    from contextlib import ExitStack

    f32 = mybir.dt.float32
    bf16 = mybir.dt.bfloat16
    f8 = mybir.dt.float8e4
    gdt = f8 if GRAM_MODE == "fp8" else bf16
    Alu = mybir.AluOpType
    Act = mybir.ActivationFunctionType
    X = mybir.AxisListType.X
    DR = mybir.MatmulPerfMode.DoubleRow if GRAM_MODE == "fp8" else None

    nc = bacc.Bacc("TRN2", target_bir_lowering=False, debug=False)

    fTq_in = nc.dram_tensor("fTq", [D, N], gdt, kind="ExternalInput").ap()
    lhq_in = nc.dram_tensor("lhq", [D, RPC], gdt, kind="ExternalInput").ap()
    frh_in = nc.dram_tensor("fold_rhs", [KF, MT * N], bf16, kind="ExternalInput").ap()
    flh_in = nc.dram_tensor("fold_lhsT", [KF, 128], bf16, kind="ExternalInput").ap()
    logits_in = nc.dram_tensor("logits", [RPC, C], bf16, kind="ExternalInput").ap()
    out_dram = nc.dram_tensor("out", [128, 16], f32, kind="ExternalOutput").ap()

    with tile.TileContext(nc) as tc, ExitStack() as ctx:
        resident = ctx.enter_context(tc.tile_pool(name="resident", bufs=1))
        psum_pool = ctx.enter_context(tc.tile_pool(name="psum", bufs=8, space="PSUM"))
        xent_pool = ctx.enter_context(tc.tile_pool(name="xent", bufs=2))
        small_pool = ctx.enter_context(tc.tile_pool(name="small", bufs=4))

        ftq = resident.tile([128, KT, N], gdt)
        lhq = resident.tile([128, KT, RPC], gdt)
        frh = resident.tile([KF, MT, N], bf16)
        flh = resident.tile([KF, 128], bf16)
        out_tile = resident.tile([128, 16], f32)
        lg = [resident.tile([128, C], bf16, tag=f"lg{r}", name=f"lg{r}") for r in range(MT)]
        mx = [resident.tile([128, NB], f32, tag=f"mx{m}", name=f"mx{m}") for m in range(MT)]
        mn = [resident.tile([128, NB], f32, tag=f"mn{m}", name=f"mn{m}") for m in range(MT)]

        # --- input DMAs; small/urgent first within each chunk step ---
        nc.sync.dma_start(flh[:], flh_in[:])
        for m in range(MT):
            nc.sync.dma_start(frh[:, m, :], frh_in[:, bass.ts(m, N)])
        for k in range(KT):
            nc.sync.dma_start(lhq[:, k, :], lhq_in[bass.ts(k, 128), :])
            nc.sync.dma_start(ftq[:, k, :], fTq_in[bass.ts(k, 128), :])
            if k < MT:
                nc.sync.dma_start(lg[k][:], logits_in[bass.ts(k, 128), :])

        # --- Gram + fold + mining ---
        for m in range(MT):
            pss = [psum_pool.tile([128, 512], f32, tag="ps", name=f"ps{m}_{n}")
                   for n in range(NB)]
            if GRAM_MODE == "fp8":
                for j in range(KT // 2):
                    w = lhq[:, 2 * j:2 * j + 2, bass.ts(m, 128)]
                    for n in range(NB):
                        nc.tensor.matmul(
                            pss[n][:], w, ftq[:, 2 * j:2 * j + 2, bass.ts(n, 512)],
                            start=(j == 0), stop=False, perf_mode=DR,
                        )
            else:
                for k in range(KT):
                    w = lhq[:, k, bass.ts(m, 128)]
                    for n in range(NB):
                        nc.tensor.matmul(
                            pss[n][:], w, ftq[:, k, bass.ts(n, 512)],
                            start=(k == 0), stop=False,
                        )
            for n in range(NB):
                nc.tensor.matmul(pss[n][:], flh[:], frh[:, m, bass.ts(n, 512)],
                                 start=False, stop=True)
            for n in range(NB):
                nc.vector.tensor_reduce(mx[m][:, n:n + 1], pss[n][:], axis=X, op=Alu.max)
                nc.vector.tensor_reduce(mn[m][:, n:n + 1], pss[n][:, bass.ts(m, 128)],
                                        axis=X, op=Alu.min)
            nc.vector.tensor_reduce(out_tile[:, m:m + 1], mx[m][:], axis=X, op=Alu.max)
            nc.vector.tensor_reduce(out_tile[:, 4 + m:5 + m], mn[m][:], axis=X, op=Alu.min)

            if m == 0:
                # xent: DVE row-max + negate, then ACT exp with accumulation
                negs = []
                for r in range(MT):
                    nc.vector.tensor_reduce(out_tile[:, 8 + r:9 + r], lg[r][:],
                                            axis=X, op=Alu.max)
                    neg = small_pool.tile([128, 1], f32, tag=f"neg{r}", name=f"neg{r}")
                    nc.vector.tensor_scalar_mul(neg[:], out_tile[:, 8 + r:9 + r], -1.0)
                    negs.append(neg)
                for r in range(MT):
                    escr = xent_pool.tile([128, C], bf16, tag="escr", name=f"escr{r}")
                    nc.scalar.activation(escr[:], lg[r][:], Act.Exp,
                                         bias=negs[r][:], scale=1.0,
                                         accum_out=out_tile[:, 12 + r:13 + r])

        nc.sync.dma_start(out_dram[:], out_tile[:])

    nc.compile()
    return nc


def _prepare(logits, feat, targets):
    logits = np.asarray(logits, dtype=np.float32)
    feat = np.asarray(feat, dtype=np.float32)
    targets = np.asarray(targets)

    perm = np.argsort(targets, kind="stable")
    t = np.asarray(targets)[perm]
    tg = t.reshape(-1, 4)
    assert (tg == tg[:, :1]).all(), "expected PK sampling with 4 instances/identity"

    feat_p = feat[perm]
    logits_p = logits[perm]

    gdt = ml_dtypes.float8_e4m3 if GRAM_MODE == "fp8" else ml_dtypes.bfloat16
    fq_small = feat_p.astype(gdt)                       # quantized [N, D]
    fq = fq_small.astype(np.float64)
    fTq = np.ascontiguousarray(fq_small.T)              # [D, N]
    sq = np.einsum("ij,ij->i", fq, fq).astype(np.float32)

    hi = sq.astype(ml_dtypes.bfloat16)
    lo = (sq.astype(np.float64) - hi.astype(np.float64)).astype(ml_dtypes.bfloat16)
    row_hi = (-0.5 * hi.astype(np.float32)).astype(ml_dtypes.bfloat16)
    row_lo = (-0.5 * lo.astype(np.float32)).astype(ml_dtypes.bfloat16)

    # fold lhsT [KF, 128]: rows 0,1 ones; row 2+g has 256 at cols 4g..4g+3
    flh = np.zeros((KF, 128), dtype=ml_dtypes.bfloat16)
    flh[0] = 1.0
    flh[1] = 1.0
    cols = np.arange(128)
    flh[2 + cols // 4, cols] = MASK_SCALE

    lgq = logits_p.astype(ml_dtypes.bfloat16)

    # target logit (host gather, matching jax clamp semantics)
    ti = t.astype(np.int64)
    ti = np.where(ti < 0, ti + C, ti)
    ti = np.clip(ti, 0, C - 1)
    tlog = logits_p[np.arange(N), ti].astype(np.float64)

    in_maps = []
    for c in range(NCORES):
        rows = slice(c * RPC, (c + 1) * RPC)
        frh = np.zeros((KF, MT * N), dtype=ml_dtypes.bfloat16)
        fr3 = frh.reshape(KF, MT, N)
        fr3[0, :, :] = row_hi[None, :]
        fr3[1, :, :] = row_lo[None, :]
        for m in range(MT):
            base = c * RPC + m * 128
            for g in range(32):
                fr3[2 + g, m, base + 4 * g: base + 4 * g + 4] = -MASK_SCALE

        in_maps.append({
            "fTq": fTq,
            "lhq": np.ascontiguousarray(fTq[:, rows]),
            "fold_rhs": frh,
            "fold_lhsT": flh,
            "logits": np.ascontiguousarray(lgq[rows]),
        })
    return in_maps, sq, tlog


def _combine(results, sq, tlog):
    outs = np.stack([r["out"].astype(np.float64) for r in results])  # [8, 128, 16]
    # global row (c, m, p) -> c*512 + m*128 + p
    mx = outs[:, :, 0:4].transpose(0, 2, 1).reshape(N)       # row max psum
    mn = outs[:, :, 4:8].transpose(0, 2, 1).reshape(N)       # row min window
    lmx = outs[:, :, 8:12].transpose(0, 2, 1).reshape(N)     # logits max
    les = outs[:, :, 12:16].transpose(0, 2, 1).reshape(N)    # exp sums

    sqd = sq.astype(np.float64)
    an2 = np.maximum(sqd - 2.0 * mx, 1e-12)
    ap2 = np.maximum(sqd - BIG - 2.0 * mn, 1e-12)
    trip = np.maximum(np.sqrt(ap2) - np.sqrt(an2) + MARGIN, 0.0)

    lse = lmx + np.log(les)
    xent = lse - tlog

    loss = ALPHA * xent.mean() + BETA * trip.mean()
    return np.float32(loss)


def kernel(logits, feat, targets):
    from concourse.bass_utils import run_bass_kernel_spmd

    if "nc" not in _compiled:
        _compiled["nc"] = _build_nc()
    nc = _compiled["nc"]

    in_maps, sq, tlog = _prepare(logits, feat, targets)
    res = run_bass_kernel_spmd(nc, in_maps, core_ids=list(range(NCORES)))
    return _combine(res.results, sq, tlog)
